# revision 1
# baseline (speedup 1.0000x reference)
"""CRF negative-log-likelihood kernel for Trainium2 (8 NeuronCores, batch-sharded).

Algorithm:
  - t2 = embedding @ fc_w computed on-device, vocab-sharded across cores (launch 1).
  - Main kernel (launch 2, batch-sharded 8 rows/core): indirect-DMA gather of
    t2 rows (16 floats/token instead of 128 -> 8x less gather traffic), PE-block
    transposes into class-on-partition layout, numerator via one-hot matmul +
    fused multiply-reduce, and a segmented forward/backward scan (L=16 steps,
    S=256 segments batched on the free dim) in linear space.
  - Host (float64, O(B*S*C) work): rank-1 junction chain across segments,
    exact partial segment for each row's ragged tail, final scalar assembly.
"""
import sys
sys.path.insert(0, "/opt/trn_rl_repo")
import numpy as np
from contextlib import ExitStack

import concourse.bass as bass
import concourse.bacc as bacc_mod
import concourse.mybir as mybir
import concourse.tile as tile
from concourse.masks import make_identity
from concourse.bass_utils import run_bass_kernel_spmd

F32 = mybir.dt.float32
I32 = mybir.dt.int32

V, E, C = 50257, 128, 16
B, T = 64, 4096
L, S = 16, 256
VPAD = 51200
VSH = VPAD // 8
BL = 8
NCHUNK = 8
CHW = T // NCHUNK
NCORES = 8

LAST_EXEC_NS = {}
_TRACE = False
_CACHE = {}


def build_t2_kernel():
    nc = bacc_mod.Bacc()
    emb_s = nc.dram_tensor("emb_s", [VSH, E], F32, kind="ExternalInput")
    fc_w = nc.dram_tensor("fc_w", [E, C], F32, kind="ExternalInput")
    t2_s = nc.dram_tensor("t2_s", [VSH, C], F32, kind="ExternalOutput")

    ntile = VSH // 128
    with ExitStack() as ctx:
        tc = ctx.enter_context(tile.TileContext(nc))
        singles = ctx.enter_context(tc.tile_pool(name="singles", bufs=1))
        psum = ctx.enter_context(tc.tile_pool(name="psum", bufs=4, space="PSUM"))

        fcw_sb = singles.tile([E, C], F32)
        nc.sync.dma_start(out=fcw_sb[:], in_=fc_w[:])
        ident = singles.tile([128, 128], F32)
        make_identity(nc, ident[:])

        # one DMA: all of emb_s, 50 blocks of (128,128) side by side
        EMB = singles.tile([128, VSH], F32)
        nc.sync.dma_start(
            out=EMB[:],
            in_=bass.AP(tensor=emb_s.handle if hasattr(emb_s, "handle") else emb_s[:].tensor,
                        offset=0, ap=[[E, 128], [128 * E, ntile], [1, E]]))
        ET = singles.tile([128, VSH], F32)
        T2 = singles.tile([128, ntile * C], F32)
        for i in range(ntile):
            psT = psum.tile([128, 128], F32, tag="pt")
            nc.tensor.transpose(psT[:], EMB[:, i * 128:(i + 1) * 128], ident[:])
            nc.vector.tensor_copy(ET[:, i * 128:(i + 1) * 128], psT[:])
        for i in range(ntile):
            ps2 = psum.tile([128, C], F32, tag="p2")
            nc.tensor.matmul(ps2[:], lhsT=ET[:, i * 128:(i + 1) * 128], rhs=fcw_sb[:],
                             start=True, stop=True)
            nc.vector.tensor_copy(T2[:, i * C:(i + 1) * C], ps2[:])
        # one DMA out: (128, ntile*C) -> t2_s (VSH, C); dst dims (r, i, j)
        nc.sync.dma_start(
            out=bass.AP(tensor=t2_s[:].tensor, offset=0,
                        ap=[[C, 128], [128 * C, ntile], [1, C]]),
            in_=T2[:])
    return nc


def _tokgather_ap(base_ap, thi):
    """Indirect-gather dest over TM tile (128, T): partition = t%128, free =
    (t//128)*128 + b*16 + j; token (b,t)'s 16 floats land contiguously.
    Partition-first enumeration (t_lo, b, j) matches the x_t index order."""
    Fd = base_ap.ap[1][1]
    return bass.AP(tensor=base_ap.tensor, offset=base_ap.offset + thi * 128,
                   ap=[[Fd, 128], [16, BL], [1, 16]])


def _strided(base_ap, k, step, count):
    return bass.AP(tensor=base_ap.tensor, offset=base_ap.offset + k,
                   ap=[base_ap.ap[0], [step, count]])


def build_main_kernel():
    nc = bacc_mod.Bacc()
    x_t = nc.dram_tensor("x_t", [128, T // 128 * BL], I32, kind="ExternalInput")
    tags_f = nc.dram_tensor("tags_f", [BL, T], F32, kind="ExternalInput")
    t2 = nc.dram_tensor("t2", [VPAD, C], F32, kind="ExternalInput")
    blockP = nc.dram_tensor("blockP", [128, 128], F32, kind="ExternalInput")
    blockPT = nc.dram_tensor("blockPT", [128, 128], F32, kind="ExternalInput")
    blockTN = nc.dram_tensor("blockTN", [128, 128], F32, kind="ExternalInput")
    bcast8 = nc.dram_tensor("bcast8", [BL, 128], F32, kind="ExternalInput")
    iota_rep = nc.dram_tensor("iota_rep", [128, CHW], F32, kind="ExternalInput")
    sadj = nc.dram_tensor("sadj", [128, 1], F32, kind="ExternalInput")

    r_out = nc.dram_tensor("r_out", [128, S], F32, kind="ExternalOutput")
    d_out = nc.dram_tensor("d_out", [128, S], F32, kind="ExternalOutput")
    num_out = nc.dram_tensor("num_out", [128, 2 * NCHUNK], F32, kind="ExternalOutput")

    with ExitStack() as ctx:
        tc = ctx.enter_context(tile.TileContext(nc))
        singles = ctx.enter_context(tc.tile_pool(name="singles", bufs=1))
        big = ctx.enter_context(tc.tile_pool(name="big", bufs=1))
        scratch = ctx.enter_context(tc.tile_pool(name="scratch", bufs=3))
        psum = ctx.enter_context(tc.tile_pool(name="psum", bufs=2, space="PSUM"))
        psum2 = ctx.enter_context(tc.tile_pool(name="psum2", bufs=1, space="PSUM"))

        xt_sb = singles.tile([128, T // 128 * BL], I32)
        nc.sync.dma_start(out=xt_sb[:], in_=x_t[:])
        tagsf_sb = singles.tile([BL, T], F32)
        nc.sync.dma_start(out=tagsf_sb[:], in_=tags_f[:])
        blockP_sb = singles.tile([128, 128], F32)
        nc.sync.dma_start(out=blockP_sb[:], in_=blockP[:])
        blockPT_sb = singles.tile([128, 128], F32)
        nc.sync.dma_start(out=blockPT_sb[:], in_=blockPT[:])
        blockTN_sb = singles.tile([128, 128], F32)
        nc.sync.dma_start(out=blockTN_sb[:], in_=blockTN[:])
        bcast8_sb = singles.tile([BL, 128], F32)
        nc.sync.dma_start(out=bcast8_sb[:], in_=bcast8[:])
        iotar_sb = singles.tile([128, CHW], F32)
        nc.sync.dma_start(out=iotar_sb[:], in_=iota_rep[:])
        sadj_sb = singles.tile([128, 1], F32)
        nc.sync.dma_start(out=sadj_sb[:], in_=sadj[:])

        TM = big.tile([128, T], F32)
        G = big.tile([128, T], F32)
        EXPG = big.tile([128, T], F32)
        W_ext = big.tile([128, T + 4], F32)
        num_sb = singles.tile([128, 2 * NCHUNK], F32)
        ident = singles.tile([128, 128], F32)
        make_identity(nc, ident[:])

        nc.vector.memset(W_ext[:, 0:1], 0.0)
        nc.vector.memset(num_sb[:], 0.0)

        TMap = TM[:]
        EXPGap = EXPG[:]

        # --- gather (token-major) + transpose blocks into G + exp ---
        for c in range(NCHUNK):
            c0 = c * CHW
            nthi = CHW // 128
            for th in range(c * nthi, (c + 1) * nthi):
                for bb in range(BL):
                    cc = th * BL + bb
                    nc.gpsimd.indirect_dma_start(
                        out=TM[:, cc * 16:(cc + 1) * 16],
                        out_offset=None,
                        in_=t2[:],
                        in_offset=bass.IndirectOffsetOnAxis(
                            ap=xt_sb[:, cc:cc + 1], axis=0),
                    )
                psT = psum.tile([128, 128], F32, tag="psT")
                nc.tensor.transpose(psT[:], TM[:, th * 128:(th + 1) * 128], ident[:])
                nc.any.tensor_copy(G[:, th * 128:(th + 1) * 128], psT[:])
            nc.scalar.activation(EXPG[:, c0:c0 + CHW], G[:, c0:c0 + CHW],
                                 mybir.ActivationFunctionType.Exp)
        nc.vector.tensor_mul(EXPG[:, 0:1], EXPG[:, 0:1], sadj_sb[:])

        # --- numerator ---
        for c in range(NCHUNK):
            c0 = c * CHW
            psA = psum.tile([128, CHW], F32, tag="ps")
            nc.tensor.matmul(psA[:], lhsT=bcast8_sb[:],
                             rhs=tagsf_sb[:, c0:c0 + CHW], start=True, stop=True)
            nc.vector.tensor_tensor(out=W_ext[:, 1 + c0:1 + c0 + CHW], in0=psA[:],
                                    in1=iotar_sb[:], op=mybir.AluOpType.is_equal)
        for c in range(NCHUNK):
            c0 = c * CHW
            psY = psum.tile([128, CHW], F32, tag="ps")
            nc.tensor.matmul(psY[:], lhsT=blockTN_sb[:],
                             rhs=W_ext[:, c0:c0 + CHW], start=True, stop=True)
            scr = scratch.tile([128, CHW], F32, tag="scr")
            nc.vector.tensor_add(scr[:], G[:, c0:c0 + CHW], psY[:])
            scr2 = scratch.tile([128, CHW], F32, tag="scr2")
            nc.vector.tensor_mul(scr2[:], scr[:], W_ext[:, 1 + c0:1 + c0 + CHW])
            nc.vector.reduce_sum(out=num_sb[:, c:c + 1], in_=scr2[:],
                                 axis=mybir.AxisListType.X)

        # --- scans ---
        r_sb = big.tile([128, S], F32)
        nc.vector.memset(r_sb[:], 1.0)
        for k in range(L):
            psR = psum2.tile([128, S], F32, tag="psR")
            nc.tensor.matmul(psR[:], lhsT=blockP_sb[:], rhs=r_sb[:],
                             start=True, stop=True)
            nc.vector.tensor_mul(r_sb[:], psR[:], _strided(EXPGap, k, L, S))

        d_sb = big.tile([128, S], F32)
        nc.vector.tensor_copy(d_sb[:], _strided(EXPGap, L - 1, L, S))
        for k in range(L - 2, -1, -1):
            psD = psum2.tile([128, S], F32, tag="psD")
            nc.tensor.matmul(psD[:], lhsT=blockPT_sb[:], rhs=d_sb[:],
                             start=True, stop=True)
            nc.vector.tensor_mul(d_sb[:], psD[:], _strided(EXPGap, k, L, S))

        nc.sync.dma_start(out=r_out[:], in_=r_sb[:])
        nc.sync.dma_start(out=d_out[:], in_=d_sb[:])
        nc.sync.dma_start(out=num_out[:], in_=num_sb[:])
    return nc


def _host_prep(embedding, fc_w, fc_b, trans, start):
    emb_pad = np.zeros((VPAD, E), np.float32)
    emb_pad[:V] = embedding
    P_eff64 = np.exp(trans.astype(np.float64) + fc_b[None, :].astype(np.float64))
    colsum = P_eff64.sum(0)
    start_adj = (np.exp(start.astype(np.float64) + fc_b) / colsum).astype(np.float32)
    trans_n = (trans + fc_b[None, :]).astype(np.float32)
    P_eff32 = P_eff64.astype(np.float32)

    eye8 = np.eye(BL, dtype=np.float32)
    return dict(
        emb_pad=emb_pad,
        P_eff=P_eff64,
        blockP=np.ascontiguousarray(np.kron(eye8, P_eff32)),
        blockPT=np.ascontiguousarray(np.kron(eye8, P_eff32.T.copy())),
        blockTN=np.ascontiguousarray(np.kron(eye8, trans_n)),
        bcast8=np.ascontiguousarray(np.kron(eye8, np.ones((1, C), np.float32))),
        iota_rep=np.ascontiguousarray(np.tile(np.tile(np.arange(C, dtype=np.float32), BL)[:, None], (1, CHW))),
        sadj=np.ascontiguousarray(np.tile(start_adj, BL)[:, None]),
    )


LAST_RESULTS = {}


def _run(nc, in_maps, label):
    res = run_bass_kernel_spmd(nc, in_maps, core_ids=list(range(NCORES)),
                               trace=_TRACE)
    if res.exec_time_ns is not None:
        LAST_EXEC_NS[label] = res.exec_time_ns
    LAST_RESULTS[label] = res
    return res.results


def kernel(x, tags, embedding, fc_w, fc_b, start_transitions, end_transitions,
           transitions):
    x = np.asarray(x, np.int32)
    tags = np.asarray(tags, np.int32)
    embedding = np.asarray(embedding, np.float32)
    fc_w = np.asarray(fc_w, np.float32)
    fc_b = np.asarray(fc_b, np.float32)
    trans = np.asarray(transitions, np.float32)
    start = np.asarray(start_transitions, np.float32)
    end = np.asarray(end_transitions, np.float32)

    prep = _host_prep(embedding, fc_w, fc_b, trans, start)

    if "t2" not in _CACHE:
        nc1 = build_t2_kernel()
        nc1.finalize()
        _CACHE["t2"] = nc1
    if "main" not in _CACHE:
        nc2 = build_main_kernel()
        nc2.finalize()
        _CACHE["main"] = nc2

    # ---- launch 1: t2 = emb_pad @ fc_w, vocab-sharded ----
    in1 = [{"emb_s": np.ascontiguousarray(prep["emb_pad"][k * VSH:(k + 1) * VSH]),
            "fc_w": fc_w} for k in range(NCORES)]
    res1 = _run(_CACHE["t2"], in1, "t2")
    t2_full = np.concatenate([res1[k]["t2_s"] for k in range(NCORES)], axis=0)
    t2_full = np.ascontiguousarray(t2_full, dtype=np.float32)

    # ---- launch 2: main kernel, batch-sharded ----
    tags_m = np.where(x != 0, tags, C).astype(np.float32)
    in2 = []
    for k in range(NCORES):
        sl = slice(k * BL, (k + 1) * BL)
        xt = x[sl].reshape(BL, T // 128, 128).transpose(2, 1, 0) \
                  .reshape(128, T // 128 * BL)
        in2.append({
            "x_t": np.ascontiguousarray(xt),
            "tags_f": np.ascontiguousarray(tags_m[sl]),
            "t2": t2_full,
            "blockP": prep["blockP"], "blockPT": prep["blockPT"],
            "blockTN": prep["blockTN"], "bcast8": prep["bcast8"],
            "iota_rep": prep["iota_rep"], "sadj": prep["sadj"],
        })
    res2 = _run(_CACHE["main"], in2, "main")

    # ---- host combine (float64) ----
    lengths = (x != 0).sum(1)
    start64 = start.astype(np.float64)
    end64 = end.astype(np.float64)
    fcb64 = fc_b.astype(np.float64)
    Pe = prep["P_eff"]
    t264 = t2_full.astype(np.float64)
    exp_end = np.exp(end64)
    total = 0.0
    for core in range(NCORES):
        num_p = np.asarray(res2[core]["num_out"], np.float64)
        r = np.asarray(res2[core]["r_out"], np.float64).reshape(BL, C, S)
        d = np.asarray(res2[core]["d_out"], np.float64).reshape(BL, C, S)
        for b in range(BL):
            gb = core * BL + b
            ln = int(lengths[gb])
            num = num_p[b * C:(b + 1) * C, :].sum()
            num += start64[tags[gb, 0]] + fcb64[tags[gb, 0]]
            num += end64[tags[gb, ln - 1]]
            sstar = (ln - 1) // L
            logZ = 0.0
            for s in range(1, sstar):
                c_s = Pe @ d[b, :, s]
                logZ += np.log(r[b, :, s - 1] @ c_s) - np.log(r[b, :, s].sum())
            alpha = r[b, :, sstar - 1].copy()
            for t in range(sstar * L, ln):
                w = np.exp(t264[x[gb, t]] + fcb64)
                alpha = (alpha @ Pe) * w
            logZ += np.log(alpha @ exp_end)
            total += -(num - logZ)
    return np.array(total, dtype=np.float32)



# revision 2
# speedup vs baseline: 3.1546x; 3.1546x over previous
"""CRF negative-log-likelihood kernel for Trainium2 (8 NeuronCores, batch-sharded).

Algorithm:
  - t2 = embedding @ fc_w computed on-device, vocab-sharded across cores (launch 1).
  - Main kernel (launch 2, batch-sharded 8 rows/core): indirect-DMA gather of
    t2 rows (16 floats/token instead of 128 -> 8x less gather traffic), PE-block
    transposes into class-on-partition layout, numerator via one-hot matmul +
    fused multiply-reduce, and a segmented forward/backward scan (L=16 steps,
    S=256 segments batched on the free dim) in linear space.
  - Host (float64, O(B*S*C) work): rank-1 junction chain across segments,
    exact partial segment for each row's ragged tail, final scalar assembly.
"""
import sys
sys.path.insert(0, "/opt/trn_rl_repo")
import numpy as np
from contextlib import ExitStack

import concourse.bass as bass
import concourse.bacc as bacc_mod
import concourse.mybir as mybir
import concourse.tile as tile
from concourse.masks import make_identity
from concourse.bass_utils import run_bass_kernel_spmd

F32 = mybir.dt.float32
I32 = mybir.dt.int32

V, E, C = 50257, 128, 16
B, T = 64, 4096
L, S = 16, 256
VPAD = 51200
VSH = VPAD // 8
BL = 8
NCHUNK = 8
CHW = T // NCHUNK
NCORES = 8

LAST_EXEC_NS = {}
_TRACE = False
_CACHE = {}


def build_t2_kernel():
    nc = bacc_mod.Bacc()
    emb_s = nc.dram_tensor("emb_s", [VSH, E], F32, kind="ExternalInput")
    fc_w = nc.dram_tensor("fc_w", [E, C], F32, kind="ExternalInput")
    t2_s = nc.dram_tensor("t2_s", [VSH, C], F32, kind="ExternalOutput")

    ntile = VSH // 128
    with ExitStack() as ctx:
        tc = ctx.enter_context(tile.TileContext(nc))
        singles = ctx.enter_context(tc.tile_pool(name="singles", bufs=1))
        psum = ctx.enter_context(tc.tile_pool(name="psum", bufs=4, space="PSUM"))

        fcw_sb = singles.tile([E, C], F32)
        nc.sync.dma_start(out=fcw_sb[:], in_=fc_w[:])
        ident = singles.tile([128, 128], F32)
        make_identity(nc, ident[:])

        # one DMA: all of emb_s, 50 blocks of (128,128) side by side
        EMB = singles.tile([128, VSH], F32)
        nc.sync.dma_start(
            out=EMB[:],
            in_=bass.AP(tensor=emb_s.handle if hasattr(emb_s, "handle") else emb_s[:].tensor,
                        offset=0, ap=[[E, 128], [128 * E, ntile], [1, E]]))
        ET = singles.tile([128, VSH], F32)
        T2 = singles.tile([128, ntile * C], F32)
        for i in range(ntile):
            psT = psum.tile([128, 128], F32, tag="pt")
            nc.tensor.transpose(psT[:], EMB[:, i * 128:(i + 1) * 128], ident[:])
            nc.vector.tensor_copy(ET[:, i * 128:(i + 1) * 128], psT[:])
        for i in range(ntile):
            ps2 = psum.tile([128, C], F32, tag="p2")
            nc.tensor.matmul(ps2[:], lhsT=ET[:, i * 128:(i + 1) * 128], rhs=fcw_sb[:],
                             start=True, stop=True)
            nc.vector.tensor_copy(T2[:, i * C:(i + 1) * C], ps2[:])
        # one DMA out: (128, ntile*C) -> t2_s (VSH, C); dst dims (r, i, j)
        nc.sync.dma_start(
            out=bass.AP(tensor=t2_s[:].tensor, offset=0,
                        ap=[[C, 128], [128 * C, ntile], [1, C]]),
            in_=T2[:])
    return nc


def _tokgather_ap(base_ap, thi):
    """Indirect-gather dest over TM tile (128, T): partition = t%128, free =
    (t//128)*128 + b*16 + j; token (b,t)'s 16 floats land contiguously.
    Partition-first enumeration (t_lo, b, j) matches the x_t index order."""
    Fd = base_ap.ap[1][1]
    return bass.AP(tensor=base_ap.tensor, offset=base_ap.offset + thi * 128,
                   ap=[[Fd, 128], [16, BL], [1, 16]])


def _strided(base_ap, k, step, count):
    return bass.AP(tensor=base_ap.tensor, offset=base_ap.offset + k,
                   ap=[base_ap.ap[0], [step, count]])


def build_main_kernel():
    nc = bacc_mod.Bacc()
    x_t = nc.dram_tensor("x_t", [128, T // 128 * BL], I32, kind="ExternalInput")
    tags_f = nc.dram_tensor("tags_f", [BL, T], F32, kind="ExternalInput")
    t2 = nc.dram_tensor("t2", [VPAD, C], F32, kind="ExternalInput")
    blockP = nc.dram_tensor("blockP", [128, 128], F32, kind="ExternalInput")
    blockPT = nc.dram_tensor("blockPT", [128, 128], F32, kind="ExternalInput")
    blockTN = nc.dram_tensor("blockTN", [128, 128], F32, kind="ExternalInput")
    bcast8 = nc.dram_tensor("bcast8", [BL, 128], F32, kind="ExternalInput")
    iota_rep = nc.dram_tensor("iota_rep", [128, CHW], F32, kind="ExternalInput")
    sadj = nc.dram_tensor("sadj", [128, 1], F32, kind="ExternalInput")

    r_out = nc.dram_tensor("r_out", [128, S], F32, kind="ExternalOutput")
    d_out = nc.dram_tensor("d_out", [128, S], F32, kind="ExternalOutput")
    num_out = nc.dram_tensor("num_out", [128, 2 * NCHUNK], F32, kind="ExternalOutput")

    with ExitStack() as ctx:
        tc = ctx.enter_context(tile.TileContext(nc))
        singles = ctx.enter_context(tc.tile_pool(name="singles", bufs=1))
        big = ctx.enter_context(tc.tile_pool(name="big", bufs=1))
        scratch = ctx.enter_context(tc.tile_pool(name="scratch", bufs=3))
        psum = ctx.enter_context(tc.tile_pool(name="psum", bufs=2, space="PSUM"))
        psum2 = ctx.enter_context(tc.tile_pool(name="psum2", bufs=1, space="PSUM"))

        xt_sb = singles.tile([128, T // 128 * BL], I32)
        nc.sync.dma_start(out=xt_sb[:], in_=x_t[:])
        tagsf_sb = singles.tile([BL, T], F32)
        nc.sync.dma_start(out=tagsf_sb[:], in_=tags_f[:])
        blockP_sb = singles.tile([128, 128], F32)
        nc.sync.dma_start(out=blockP_sb[:], in_=blockP[:])
        blockPT_sb = singles.tile([128, 128], F32)
        nc.sync.dma_start(out=blockPT_sb[:], in_=blockPT[:])
        blockTN_sb = singles.tile([128, 128], F32)
        nc.sync.dma_start(out=blockTN_sb[:], in_=blockTN[:])
        bcast8_sb = singles.tile([BL, 128], F32)
        nc.sync.dma_start(out=bcast8_sb[:], in_=bcast8[:])
        iotar_sb = singles.tile([128, CHW], F32)
        nc.sync.dma_start(out=iotar_sb[:], in_=iota_rep[:])
        sadj_sb = singles.tile([128, 1], F32)
        nc.sync.dma_start(out=sadj_sb[:], in_=sadj[:])

        TM = big.tile([128, T], F32)
        G = big.tile([128, T], F32)
        EXPG = big.tile([128, T], F32)
        W_ext = big.tile([128, T + 4], F32)
        num_sb = singles.tile([128, 2 * NCHUNK], F32)
        ident = singles.tile([128, 128], F32)
        make_identity(nc, ident[:])

        nc.vector.memset(W_ext[:, 0:1], 0.0)
        nc.vector.memset(num_sb[:], 0.0)

        TMap = TM[:]
        EXPGap = EXPG[:]

        # --- gather (token-major) + transpose blocks into G + exp ---
        # One merged indirect DMA per chunk: offsets xt_sb[:, c*32:(c+1)*32]
        # enumerate (partition, col) C-order; each offset owns 16 contiguous
        # floats of the dest view — identical mapping to 256 per-column calls
        # but amortizes the ~1us SWDGE fixed cost per call.
        nthi = CHW // 128
        ncc = CHW // 16
        for c in range(NCHUNK):
            c0 = c * CHW
            nc.gpsimd.indirect_dma_start(
                out=TM[:, c0:c0 + CHW],
                out_offset=None,
                in_=t2[:],
                in_offset=bass.IndirectOffsetOnAxis(
                    ap=xt_sb[:, c * ncc:(c + 1) * ncc], axis=0),
            )
        for c in range(NCHUNK):
            c0 = c * CHW
            for th in range(c * nthi, (c + 1) * nthi):
                psT = psum.tile([128, 128], F32, tag="psT")
                nc.tensor.transpose(psT[:], TM[:, th * 128:(th + 1) * 128], ident[:])
                nc.any.tensor_copy(G[:, th * 128:(th + 1) * 128], psT[:])
            nc.scalar.activation(EXPG[:, c0:c0 + CHW], G[:, c0:c0 + CHW],
                                 mybir.ActivationFunctionType.Exp)
        nc.vector.tensor_mul(EXPG[:, 0:1], EXPG[:, 0:1], sadj_sb[:])

        # --- numerator ---
        for c in range(NCHUNK):
            c0 = c * CHW
            psA = psum.tile([128, CHW], F32, tag="ps")
            nc.tensor.matmul(psA[:], lhsT=bcast8_sb[:],
                             rhs=tagsf_sb[:, c0:c0 + CHW], start=True, stop=True)
            nc.vector.tensor_tensor(out=W_ext[:, 1 + c0:1 + c0 + CHW], in0=psA[:],
                                    in1=iotar_sb[:], op=mybir.AluOpType.is_equal)
        for c in range(NCHUNK):
            c0 = c * CHW
            psY = psum.tile([128, CHW], F32, tag="ps")
            nc.tensor.matmul(psY[:], lhsT=blockTN_sb[:],
                             rhs=W_ext[:, c0:c0 + CHW], start=True, stop=True)
            scr = scratch.tile([128, CHW], F32, tag="scr")
            nc.vector.tensor_add(scr[:], G[:, c0:c0 + CHW], psY[:])
            scr2 = scratch.tile([128, CHW], F32, tag="scr2")
            nc.vector.tensor_mul(scr2[:], scr[:], W_ext[:, 1 + c0:1 + c0 + CHW])
            nc.vector.reduce_sum(out=num_sb[:, c:c + 1], in_=scr2[:],
                                 axis=mybir.AxisListType.X)

        # --- scans ---
        r_sb = big.tile([128, S], F32)
        nc.vector.memset(r_sb[:], 1.0)
        for k in range(L):
            psR = psum2.tile([128, S], F32, tag="psR")
            nc.tensor.matmul(psR[:], lhsT=blockP_sb[:], rhs=r_sb[:],
                             start=True, stop=True)
            nc.vector.tensor_mul(r_sb[:], psR[:], _strided(EXPGap, k, L, S))

        d_sb = big.tile([128, S], F32)
        nc.vector.tensor_copy(d_sb[:], _strided(EXPGap, L - 1, L, S))
        for k in range(L - 2, -1, -1):
            psD = psum2.tile([128, S], F32, tag="psD")
            nc.tensor.matmul(psD[:], lhsT=blockPT_sb[:], rhs=d_sb[:],
                             start=True, stop=True)
            nc.vector.tensor_mul(d_sb[:], psD[:], _strided(EXPGap, k, L, S))

        nc.sync.dma_start(out=r_out[:], in_=r_sb[:])
        nc.sync.dma_start(out=d_out[:], in_=d_sb[:])
        nc.sync.dma_start(out=num_out[:], in_=num_sb[:])
    return nc


def _host_prep(embedding, fc_w, fc_b, trans, start):
    emb_pad = np.zeros((VPAD, E), np.float32)
    emb_pad[:V] = embedding
    P_eff64 = np.exp(trans.astype(np.float64) + fc_b[None, :].astype(np.float64))
    colsum = P_eff64.sum(0)
    start_adj = (np.exp(start.astype(np.float64) + fc_b) / colsum).astype(np.float32)
    trans_n = (trans + fc_b[None, :]).astype(np.float32)
    P_eff32 = P_eff64.astype(np.float32)

    eye8 = np.eye(BL, dtype=np.float32)
    return dict(
        emb_pad=emb_pad,
        P_eff=P_eff64,
        blockP=np.ascontiguousarray(np.kron(eye8, P_eff32)),
        blockPT=np.ascontiguousarray(np.kron(eye8, P_eff32.T.copy())),
        blockTN=np.ascontiguousarray(np.kron(eye8, trans_n)),
        bcast8=np.ascontiguousarray(np.kron(eye8, np.ones((1, C), np.float32))),
        iota_rep=np.ascontiguousarray(np.tile(np.tile(np.arange(C, dtype=np.float32), BL)[:, None], (1, CHW))),
        sadj=np.ascontiguousarray(np.tile(start_adj, BL)[:, None]),
    )


LAST_RESULTS = {}


def _run(nc, in_maps, label):
    res = run_bass_kernel_spmd(nc, in_maps, core_ids=list(range(NCORES)),
                               trace=_TRACE)
    if res.exec_time_ns is not None:
        LAST_EXEC_NS[label] = res.exec_time_ns
    LAST_RESULTS[label] = res
    return res.results


def kernel(x, tags, embedding, fc_w, fc_b, start_transitions, end_transitions,
           transitions):
    x = np.asarray(x, np.int32)
    tags = np.asarray(tags, np.int32)
    embedding = np.asarray(embedding, np.float32)
    fc_w = np.asarray(fc_w, np.float32)
    fc_b = np.asarray(fc_b, np.float32)
    trans = np.asarray(transitions, np.float32)
    start = np.asarray(start_transitions, np.float32)
    end = np.asarray(end_transitions, np.float32)

    prep = _host_prep(embedding, fc_w, fc_b, trans, start)

    if "t2" not in _CACHE:
        nc1 = build_t2_kernel()
        nc1.finalize()
        _CACHE["t2"] = nc1
    if "main" not in _CACHE:
        nc2 = build_main_kernel()
        nc2.finalize()
        _CACHE["main"] = nc2

    # ---- launch 1: t2 = emb_pad @ fc_w, vocab-sharded ----
    in1 = [{"emb_s": np.ascontiguousarray(prep["emb_pad"][k * VSH:(k + 1) * VSH]),
            "fc_w": fc_w} for k in range(NCORES)]
    res1 = _run(_CACHE["t2"], in1, "t2")
    t2_full = np.concatenate([res1[k]["t2_s"] for k in range(NCORES)], axis=0)
    t2_full = np.ascontiguousarray(t2_full, dtype=np.float32)

    # ---- launch 2: main kernel, batch-sharded ----
    tags_m = np.where(x != 0, tags, C).astype(np.float32)
    in2 = []
    for k in range(NCORES):
        sl = slice(k * BL, (k + 1) * BL)
        xt = x[sl].reshape(BL, T // 128, 128).transpose(2, 1, 0) \
                  .reshape(128, T // 128 * BL)
        in2.append({
            "x_t": np.ascontiguousarray(xt),
            "tags_f": np.ascontiguousarray(tags_m[sl]),
            "t2": t2_full,
            "blockP": prep["blockP"], "blockPT": prep["blockPT"],
            "blockTN": prep["blockTN"], "bcast8": prep["bcast8"],
            "iota_rep": prep["iota_rep"], "sadj": prep["sadj"],
        })
    res2 = _run(_CACHE["main"], in2, "main")

    # ---- host combine (float64) ----
    lengths = (x != 0).sum(1)
    start64 = start.astype(np.float64)
    end64 = end.astype(np.float64)
    fcb64 = fc_b.astype(np.float64)
    Pe = prep["P_eff"]
    t264 = t2_full.astype(np.float64)
    exp_end = np.exp(end64)
    total = 0.0
    for core in range(NCORES):
        num_p = np.asarray(res2[core]["num_out"], np.float64)
        r = np.asarray(res2[core]["r_out"], np.float64).reshape(BL, C, S)
        d = np.asarray(res2[core]["d_out"], np.float64).reshape(BL, C, S)
        for b in range(BL):
            gb = core * BL + b
            ln = int(lengths[gb])
            num = num_p[b * C:(b + 1) * C, :].sum()
            num += start64[tags[gb, 0]] + fcb64[tags[gb, 0]]
            num += end64[tags[gb, ln - 1]]
            sstar = (ln - 1) // L
            logZ = 0.0
            for s in range(1, sstar):
                c_s = Pe @ d[b, :, s]
                logZ += np.log(r[b, :, s - 1] @ c_s) - np.log(r[b, :, s].sum())
            alpha = r[b, :, sstar - 1].copy()
            for t in range(sstar * L, ln):
                w = np.exp(t264[x[gb, t]] + fcb64)
                alpha = (alpha @ Pe) * w
            logZ += np.log(alpha @ exp_end)
            total += -(num - logZ)
    return np.array(total, dtype=np.float32)



# revision 12
# speedup vs baseline: 4.7906x; 1.5186x over previous
"""CRF negative-log-likelihood kernel for Trainium2 (8 NeuronCores, batch-sharded).

Algorithm:
  - Launch 1 (vocab-sharded): t2 = embedding @ fc_w in bf16. Host pre-transposes
    the embedding shard so the kernel is just convert-to-bf16 + 50 matmuls
    (lhsT = embT chunk, rhs = fc_w), no PE transposes. Output t2 is bf16
    (32B rows) to halve gather traffic.
  - Launch 2 (batch-sharded, 8 rows/core, bf16 compute): merged indirect-DMA
    gathers of t2 rows (8 calls, 4096 descriptors each), bf16 PE-block
    transposes into class-on-partition layout, numerator via one-hot matmul +
    multiply-reduce, and a segmented forward/backward scan (L=16 steps, S=256
    segments on the free dim) in linear space with the two scan chains
    interleaved so vector muls hide behind the other chain's matmuls.
  - Host (float64, vectorized): rank-1 junction chain across segments, exact
    partial segment for each row's ragged tail, final scalar assembly.
"""
import sys
sys.path.insert(0, "/opt/trn_rl_repo")
import numpy as np
import ml_dtypes
from contextlib import ExitStack

import concourse.bass as bass
import concourse.bacc as bacc_mod
import concourse.mybir as mybir
import concourse.tile as tile
from concourse.masks import make_identity
from concourse.bass_utils import run_bass_kernel_spmd

F32 = mybir.dt.float32
BF16 = mybir.dt.bfloat16
I32 = mybir.dt.int32
NPBF = ml_dtypes.bfloat16

V, E, C = 50257, 128, 16
B, T = 64, 4096
L, S = 16, 256
VPAD = 51200
VSH = VPAD // 8
BL = 8
NCHUNK = 8
CHW = T // NCHUNK
NCORES = 8

LAST_EXEC_NS = {}
_TRACE = False
_CACHE = {}


def build_t2_kernel():
    nc = bacc_mod.Bacc()
    # embT_s: host-pretransposed shard, (E, VSH) f32
    embT_s = nc.dram_tensor("embT_s", [E, VSH], F32, kind="ExternalInput")
    fc_w = nc.dram_tensor("fc_w", [E, C], F32, kind="ExternalInput")
    t2_s = nc.dram_tensor("t2_s", [VSH, C], BF16, kind="ExternalOutput")

    ntile = VSH // 128          # 50 chunks of 128 vocab rows
    NGRP = 10                   # DMA/convert granularity: 5 chunks per group
    GW = VSH // NGRP            # 640 columns per group
    with ExitStack() as ctx:
        tc = ctx.enter_context(tile.TileContext(nc))
        singles = ctx.enter_context(tc.tile_pool(name="singles", bufs=1))
        psum = ctx.enter_context(tc.tile_pool(name="psum", bufs=4, space="PSUM"))

        fcw_f32 = singles.tile([E, C], F32)
        nc.sync.dma_start(out=fcw_f32[:], in_=fc_w[:])
        fcw_bf = singles.tile([E, C], BF16)
        nc.vector.tensor_copy(fcw_bf[:], fcw_f32[:])

        EMBT = singles.tile([128, VSH], F32)
        EMBTb = singles.tile([128, VSH], BF16)
        T2 = singles.tile([128, ntile * C], BF16)
        for g in range(NGRP):
            nc.sync.dma_start(out=EMBT[:, g * GW:(g + 1) * GW],
                              in_=embT_s[:, g * GW:(g + 1) * GW])
            if g % 2 == 0:
                nc.vector.tensor_copy(EMBTb[:, g * GW:(g + 1) * GW],
                                      EMBT[:, g * GW:(g + 1) * GW])
            else:
                nc.scalar.copy(EMBTb[:, g * GW:(g + 1) * GW],
                               EMBT[:, g * GW:(g + 1) * GW])
        for i in range(ntile):
            ps2 = psum.tile([128, C], F32, tag="p2")
            nc.tensor.matmul(ps2[:], lhsT=EMBTb[:, i * 128:(i + 1) * 128],
                             rhs=fcw_bf[:], start=True, stop=True)
            if i % 2 == 0:
                nc.vector.tensor_copy(T2[:, i * C:(i + 1) * C], ps2[:])
            else:
                nc.scalar.copy(T2[:, i * C:(i + 1) * C], ps2[:])
        # one DMA out: (128, ntile*C) -> t2_s (VSH, C); dst dims (r, i, j)
        nc.sync.dma_start(
            out=bass.AP(tensor=t2_s[:].tensor, offset=0,
                        ap=[[C, 128], [128 * C, ntile], [1, C]]),
            in_=T2[:])
    return nc


def _strided(base_ap, k, step, count):
    return bass.AP(tensor=base_ap.tensor, offset=base_ap.offset + k,
                   ap=[base_ap.ap[0], [step, count]])


def build_main_kernel():
    nc = bacc_mod.Bacc()
    x_t = nc.dram_tensor("x_t", [128, T // 128 * BL], I32, kind="ExternalInput")
    tags_f = nc.dram_tensor("tags_f", [BL, T], BF16, kind="ExternalInput")
    t2 = nc.dram_tensor("t2", [VPAD, C], BF16, kind="ExternalInput")
    blockP = nc.dram_tensor("blockP", [128, 128], BF16, kind="ExternalInput")
    blockPT = nc.dram_tensor("blockPT", [128, 128], BF16, kind="ExternalInput")
    bcast8 = nc.dram_tensor("bcast8", [BL, 128], BF16, kind="ExternalInput")
    iota_rep = nc.dram_tensor("iota_rep", [128, CHW], BF16, kind="ExternalInput")
    sadj = nc.dram_tensor("sadj", [128, 1], BF16, kind="ExternalInput")

    r_out = nc.dram_tensor("r_out", [128, S], BF16, kind="ExternalOutput")
    d_out = nc.dram_tensor("d_out", [128, S], BF16, kind="ExternalOutput")
    num_out = nc.dram_tensor("num_out", [128, 2 * NCHUNK], F32, kind="ExternalOutput")

    with ExitStack() as ctx:
        tc = ctx.enter_context(tile.TileContext(nc))
        singles = ctx.enter_context(tc.tile_pool(name="singles", bufs=1))
        big = ctx.enter_context(tc.tile_pool(name="big", bufs=1))
        scratch = ctx.enter_context(tc.tile_pool(name="scratch", bufs=3))
        psum = ctx.enter_context(tc.tile_pool(name="psum", bufs=2, space="PSUM"))
        psum2 = ctx.enter_context(tc.tile_pool(name="psum2", bufs=2, space="PSUM"))

        # input DMAs spread across queues for parallel issue
        xt_sb = singles.tile([128, T // 128 * BL], I32)
        nc.sync.dma_start(out=xt_sb[:], in_=x_t[:])
        tagsf_sb = singles.tile([BL, T], BF16)
        nc.scalar.dma_start(out=tagsf_sb[:], in_=tags_f[:])
        bcast8_sb = singles.tile([BL, 128], BF16)
        nc.scalar.dma_start(out=bcast8_sb[:], in_=bcast8[:])
        iotar_sb = singles.tile([128, CHW], BF16)
        nc.scalar.dma_start(out=iotar_sb[:], in_=iota_rep[:])
        blockP_sb = singles.tile([128, 128], BF16)
        nc.gpsimd.dma_start(out=blockP_sb[:], in_=blockP[:])
        blockPT_sb = singles.tile([128, 128], BF16)
        nc.gpsimd.dma_start(out=blockPT_sb[:], in_=blockPT[:])
        sadj_sb = singles.tile([128, 1], BF16)
        nc.gpsimd.dma_start(out=sadj_sb[:], in_=sadj[:])

        TM = big.tile([128, T], BF16)
        G = big.tile([128, T], BF16)
        EXPG = big.tile([128, T], BF16)
        W = big.tile([128, T], BF16)
        num_sb = singles.tile([128, 2 * NCHUNK], F32)

        nc.vector.memset(num_sb[:], 0.0)

        EXPGap = EXPG[:]
        Gap = G[:]

        # --- numerator W build first: only needs tags, runs during gather ---
        for c in range(NCHUNK):
            c0 = c * CHW
            psA = psum.tile([128, CHW], F32, tag="ps")
            nc.tensor.matmul(psA[:], lhsT=bcast8_sb[:],
                             rhs=tagsf_sb[:, c0:c0 + CHW], start=True, stop=True)
            nc.vector.tensor_tensor(out=W[:, c0:c0 + CHW], in0=psA[:],
                                    in1=iotar_sb[:], op=mybir.AluOpType.is_equal)

        # --- gather (token-major): one merged indirect DMA per chunk.
        # Offsets xt_sb[:, c*32:(c+1)*32] enumerate (partition, col) C-order;
        # each offset owns 16 contiguous bf16 of the dest view — identical
        # mapping to per-column calls but amortizes the ~1us SWDGE fixed cost.
        ncc = CHW // 16
        for c in range(NCHUNK):
            c0 = c * CHW
            nc.gpsimd.indirect_dma_start(
                out=TM[:, c0:c0 + CHW],
                out_offset=None,
                in_=t2[:],
                in_offset=bass.IndirectOffsetOnAxis(
                    ap=xt_sb[:, c * ncc:(c + 1) * ncc], axis=0),
            )

        # --- XBAR block-transpose TM -> G (2 chunks per call) + exp ---
        # out view (128, nb, 128): out[p, b, j] = TM[j, g0 + b*128 + p],
        # i.e. an independent transpose of each 128x128 block.
        GRPW = 2 * CHW
        nbl = GRPW // 128
        for g in range(NCHUNK // 2):
            g0 = g * GRPW
            eng = nc.sync if g % 2 == 0 else nc.scalar
            eng.dma_start_transpose(
                out=bass.AP(tensor=Gap.tensor, offset=g0,
                            ap=[[Gap.ap[0][0], 128], [128, nbl], [1, 128]]),
                in_=TM[:, g0:g0 + GRPW])
            nc.scalar.activation(EXPG[:, g0:g0 + GRPW], G[:, g0:g0 + GRPW],
                                 mybir.ActivationFunctionType.Exp)
        nc.vector.tensor_mul(EXPG[:, 0:1], EXPG[:, 0:1], sadj_sb[:])

        # --- numerator: em_tag only (transition terms done on host) ---
        for c in range(NCHUNK):
            c0 = c * CHW
            scr2 = scratch.tile([128, CHW], BF16, tag="scr2")
            nc.vector.tensor_mul(scr2[:], G[:, c0:c0 + CHW], W[:, c0:c0 + CHW])
            nc.vector.reduce_sum(out=num_sb[:, c:c + 1], in_=scr2[:],
                                 axis=mybir.AxisListType.X)

        # --- scans: forward and backward chains interleaved ---
        r_sb = big.tile([128, S], BF16)
        nc.vector.memset(r_sb[:], 1.0)
        d_sb = big.tile([128, S], BF16)
        nc.vector.tensor_copy(d_sb[:], _strided(EXPGap, L - 1, L, S))
        for i in range(L):
            kf = i               # forward step k = 0..15
            kb = L - 2 - i       # backward step k = 14..0
            psR = psum2.tile([128, S], F32, tag="psR")
            nc.tensor.matmul(psR[:], lhsT=blockP_sb[:], rhs=r_sb[:],
                             start=True, stop=True)
            if kb >= 0:
                psD = psum2.tile([128, S], F32, tag="psD")
                nc.tensor.matmul(psD[:], lhsT=blockPT_sb[:], rhs=d_sb[:],
                                 start=True, stop=True)
            nc.vector.tensor_mul(r_sb[:], psR[:], _strided(EXPGap, kf, L, S))
            if kb >= 0:
                nc.vector.tensor_mul(d_sb[:], psD[:], _strided(EXPGap, kb, L, S))

        nc.sync.dma_start(out=r_out[:], in_=r_sb[:])
        nc.sync.dma_start(out=d_out[:], in_=d_sb[:])
        nc.sync.dma_start(out=num_out[:], in_=num_sb[:])
    return nc


def _host_prep(embedding, fc_w, fc_b, trans, start):
    P_eff64 = np.exp(trans.astype(np.float64) + fc_b[None, :].astype(np.float64))
    colsum = P_eff64.sum(0)
    start_adj = (np.exp(start.astype(np.float64) + fc_b) / colsum).astype(np.float32)
    trans_n = (trans + fc_b[None, :]).astype(np.float32)
    P_eff32 = P_eff64.astype(np.float32)

    eye8 = np.eye(BL, dtype=np.float32)
    return dict(
        P_eff=P_eff64,
        trans_n=trans_n.astype(np.float64),
        blockP=np.ascontiguousarray(np.kron(eye8, P_eff32)).astype(NPBF),
        blockPT=np.ascontiguousarray(np.kron(eye8, P_eff32.T.copy())).astype(NPBF),
        bcast8=np.ascontiguousarray(np.kron(eye8, np.ones((1, C), np.float32))).astype(NPBF),
        iota_rep=np.ascontiguousarray(
            np.tile(np.tile(np.arange(C, dtype=np.float32), BL)[:, None],
                    (1, CHW))).astype(NPBF),
        sadj=np.ascontiguousarray(np.tile(start_adj, BL)[:, None]).astype(NPBF),
    )


LAST_RESULTS = {}


def _run(nc, in_maps, label):
    res = run_bass_kernel_spmd(nc, in_maps, core_ids=list(range(NCORES)),
                               trace=_TRACE)
    if res.exec_time_ns is not None:
        LAST_EXEC_NS[label] = res.exec_time_ns
    LAST_RESULTS[label] = res
    return res.results


def kernel(x, tags, embedding, fc_w, fc_b, start_transitions, end_transitions,
           transitions):
    x = np.asarray(x, np.int32)
    tags = np.asarray(tags, np.int32)
    embedding = np.asarray(embedding, np.float32)
    fc_w = np.asarray(fc_w, np.float32)
    fc_b = np.asarray(fc_b, np.float32)
    trans = np.asarray(transitions, np.float32)
    start = np.asarray(start_transitions, np.float32)
    end = np.asarray(end_transitions, np.float32)

    prep = _host_prep(embedding, fc_w, fc_b, trans, start)

    if "t2" not in _CACHE:
        nc1 = build_t2_kernel()
        nc1.finalize()
        _CACHE["t2"] = nc1
    if "main" not in _CACHE:
        nc2 = build_main_kernel()
        nc2.finalize()
        _CACHE["main"] = nc2

    # ---- launch 1: t2 = emb_pad @ fc_w (bf16 out), vocab-sharded ----
    emb_pad_T = np.zeros((E, VPAD), np.float32)
    emb_pad_T[:, :V] = embedding.T
    in1 = [{"embT_s": np.ascontiguousarray(emb_pad_T[:, k * VSH:(k + 1) * VSH]),
            "fc_w": fc_w} for k in range(NCORES)]
    res1 = _run(_CACHE["t2"], in1, "t2")
    t2_full = np.concatenate([np.asarray(res1[k]["t2_s"]) for k in range(NCORES)],
                             axis=0)
    t2_full = np.ascontiguousarray(t2_full)          # (VPAD, C) bf16

    # ---- launch 2: main kernel, batch-sharded ----
    tags_m = np.where(x != 0, tags, C).astype(NPBF)
    in2 = []
    for k in range(NCORES):
        sl = slice(k * BL, (k + 1) * BL)
        xt = x[sl].reshape(BL, T // 128, 128).transpose(2, 1, 0) \
                  .reshape(128, T // 128 * BL)
        in2.append({
            "x_t": np.ascontiguousarray(xt),
            "tags_f": np.ascontiguousarray(tags_m[sl]),
            "t2": t2_full,
            "blockP": prep["blockP"], "blockPT": prep["blockPT"],
            "bcast8": prep["bcast8"],
            "iota_rep": prep["iota_rep"], "sadj": prep["sadj"],
        })
    res2 = _run(_CACHE["main"], in2, "main")

    # ---- host combine (float64, vectorized) ----
    lengths = (x != 0).sum(1)                        # (B,)
    start64 = start.astype(np.float64)
    end64 = end.astype(np.float64)
    fcb64 = fc_b.astype(np.float64)
    Pe = prep["P_eff"]                               # (C, C) float64
    t264 = t2_full.astype(np.float64)                # (VPAD, C)
    exp_end = np.exp(end64)

    num_p = np.concatenate(
        [np.asarray(res2[k]["num_out"], np.float64) for k in range(NCORES)],
        axis=0).reshape(B, C, 2 * NCHUNK)            # (B, C, 16)
    r = np.concatenate(
        [np.asarray(res2[k]["r_out"]).astype(np.float64).reshape(BL, C, S)
         for k in range(NCORES)], axis=0)            # (B, C, S)
    d = np.concatenate(
        [np.asarray(res2[k]["d_out"]).astype(np.float64).reshape(BL, C, S)
         for k in range(NCORES)], axis=0)            # (B, C, S)

    num = num_p.sum(axis=(1, 2))
    num += start64[tags[:, 0]] + fcb64[tags[:, 0]]
    num += end64[tags[np.arange(B), lengths - 1]]
    # transition terms (pure tags/params, no device data)
    maskf = (x[:, 1:] != 0).astype(np.float64)
    num += (prep["trans_n"][tags[:, :-1], tags[:, 1:]] * maskf).sum(axis=1)

    # full-segment junction chain: for s in 1..sstar-1:
    #   logZ += log(r[:,:,s-1] @ (Pe @ d[:,:,s])) - log(r[:,:,s].sum())
    sstar = (lengths - 1) // L                       # (B,)
    cs = np.einsum('cd,bds->bcs', Pe, d)             # (B, C, S)
    t1 = np.einsum('bcs,bcs->bs', r[:, :, :-1], cs[:, :, 1:])   # junction at s=1..S-1
    rs = r.sum(axis=1)                               # (B, S)
    s_idx = np.arange(1, S)[None, :]                 # (1, S-1)
    jmask = s_idx <= (sstar[:, None] - 1)            # (B, S-1)
    terms = np.where(jmask, np.log(t1) - np.log(rs[:, 1:]), 0.0)
    logZ = terms.sum(axis=1)                         # (B,)

    # ragged tail: exact alpha recursion from segment sstar-1's r
    alpha = r[np.arange(B), :, sstar - 1].copy()     # (B, C)
    tail_len = lengths - sstar * L                   # in [1, L]
    for t_off in range(L):
        active = t_off < tail_len                    # (B,)
        t_idx = np.minimum(sstar * L + t_off, T - 1)
        w = np.exp(t264[x[np.arange(B), t_idx]] + fcb64[None, :])   # (B, C)
        alpha_new = (alpha @ Pe) * w
        alpha = np.where(active[:, None], alpha_new, alpha)
    logZ += np.log(alpha @ exp_end)

    total = -(num - logZ).sum()
    return np.array(total, dtype=np.float32)


# revision 16
# speedup vs baseline: 4.8154x; 1.0052x over previous
"""CRF negative-log-likelihood kernel for Trainium2 (8 NeuronCores, batch-sharded).

Algorithm:
  - Launch 1 (vocab-sharded): t2 = embedding @ fc_w in bf16. Host pre-transposes
    the embedding shard so the kernel is just convert-to-bf16 + 50 matmuls
    (lhsT = embT chunk, rhs = fc_w), no PE transposes. Output t2 is bf16
    (32B rows) to halve gather traffic.
  - Launch 2 (batch-sharded, 8 rows/core, bf16 compute): merged indirect-DMA
    gathers of t2 rows (8 calls, 4096 descriptors each), bf16 PE-block
    transposes into class-on-partition layout, numerator via one-hot matmul +
    multiply-reduce, and a segmented forward/backward scan (L=16 steps, S=256
    segments on the free dim) in linear space with the two scan chains
    interleaved so vector muls hide behind the other chain's matmuls.
  - Host (float64, vectorized): rank-1 junction chain across segments, exact
    partial segment for each row's ragged tail, final scalar assembly.
"""
import sys
sys.path.insert(0, "/opt/trn_rl_repo")
import numpy as np
import ml_dtypes
from contextlib import ExitStack

import concourse.bass as bass
import concourse.bacc as bacc_mod
import concourse.mybir as mybir
import concourse.tile as tile
from concourse.masks import make_identity
from concourse.bass_utils import run_bass_kernel_spmd

F32 = mybir.dt.float32
BF16 = mybir.dt.bfloat16
I32 = mybir.dt.int32
NPBF = ml_dtypes.bfloat16

V, E, C = 50257, 128, 16
B, T = 64, 4096
L, S = 8, 512
VPAD = 51200
VSH = VPAD // 8
BL = 8
NCHUNK = 8
CHW = T // NCHUNK
NCORES = 8

LAST_EXEC_NS = {}
_TRACE = False
_CACHE = {}


def build_t2_kernel():
    nc = bacc_mod.Bacc()
    # embT_s: host-pretransposed shard, (E, VSH) f32
    embT_s = nc.dram_tensor("embT_s", [E, VSH], F32, kind="ExternalInput")
    fc_w = nc.dram_tensor("fc_w", [E, C], F32, kind="ExternalInput")
    t2_s = nc.dram_tensor("t2_s", [VSH, C], BF16, kind="ExternalOutput")

    ntile = VSH // 128          # 50 chunks of 128 vocab rows
    NGRP = 10                   # DMA/convert granularity: 5 chunks per group
    GW = VSH // NGRP            # 640 columns per group
    with ExitStack() as ctx:
        tc = ctx.enter_context(tile.TileContext(nc))
        singles = ctx.enter_context(tc.tile_pool(name="singles", bufs=1))
        psum = ctx.enter_context(tc.tile_pool(name="psum", bufs=4, space="PSUM"))

        fcw_f32 = singles.tile([E, C], F32)
        nc.sync.dma_start(out=fcw_f32[:], in_=fc_w[:])
        fcw_bf = singles.tile([E, C], BF16)
        nc.vector.tensor_copy(fcw_bf[:], fcw_f32[:])

        EMBT = singles.tile([128, VSH], F32)
        EMBTb = singles.tile([128, VSH], BF16)
        T2 = singles.tile([128, ntile * C], BF16)
        for g in range(NGRP):
            eng = nc.sync if g % 2 == 0 else nc.gpsimd
            eng.dma_start(out=EMBT[:, g * GW:(g + 1) * GW],
                          in_=embT_s[:, g * GW:(g + 1) * GW])
            if g % 2 == 0:
                nc.vector.tensor_copy(EMBTb[:, g * GW:(g + 1) * GW],
                                      EMBT[:, g * GW:(g + 1) * GW])
            else:
                nc.scalar.copy(EMBTb[:, g * GW:(g + 1) * GW],
                               EMBT[:, g * GW:(g + 1) * GW])
        for i in range(ntile):
            ps2 = psum.tile([128, C], F32, tag="p2")
            nc.tensor.matmul(ps2[:], lhsT=EMBTb[:, i * 128:(i + 1) * 128],
                             rhs=fcw_bf[:], start=True, stop=True)
            if i % 2 == 0:
                nc.vector.tensor_copy(T2[:, i * C:(i + 1) * C], ps2[:])
            else:
                nc.scalar.copy(T2[:, i * C:(i + 1) * C], ps2[:])
        # one DMA out: (128, ntile*C) -> t2_s (VSH, C); dst dims (r, i, j)
        nc.sync.dma_start(
            out=bass.AP(tensor=t2_s[:].tensor, offset=0,
                        ap=[[C, 128], [128 * C, ntile], [1, C]]),
            in_=T2[:])
    return nc


def _strided(base_ap, k, step, count):
    return bass.AP(tensor=base_ap.tensor, offset=base_ap.offset + k,
                   ap=[base_ap.ap[0], [step, count]])


def build_main_kernel():
    nc = bacc_mod.Bacc()
    x_t = nc.dram_tensor("x_t", [128, T // 128 * BL], I32, kind="ExternalInput")
    tags_f = nc.dram_tensor("tags_f", [BL, T], BF16, kind="ExternalInput")
    t2 = nc.dram_tensor("t2", [VPAD, C], BF16, kind="ExternalInput")
    blockP = nc.dram_tensor("blockP", [128, 128], BF16, kind="ExternalInput")
    blockPT = nc.dram_tensor("blockPT", [128, 128], BF16, kind="ExternalInput")
    bcast8 = nc.dram_tensor("bcast8", [BL, 128], BF16, kind="ExternalInput")
    iota_rep = nc.dram_tensor("iota_rep", [128, CHW], BF16, kind="ExternalInput")
    sadj = nc.dram_tensor("sadj", [128, 1], BF16, kind="ExternalInput")

    r_out = nc.dram_tensor("r_out", [128, S], BF16, kind="ExternalOutput")
    d_out = nc.dram_tensor("d_out", [128, S], BF16, kind="ExternalOutput")
    num_out = nc.dram_tensor("num_out", [128, 2 * NCHUNK], F32, kind="ExternalOutput")

    with ExitStack() as ctx:
        tc = ctx.enter_context(tile.TileContext(nc))
        singles = ctx.enter_context(tc.tile_pool(name="singles", bufs=1))
        big = ctx.enter_context(tc.tile_pool(name="big", bufs=1))
        scratch = ctx.enter_context(tc.tile_pool(name="scratch", bufs=3))
        psum = ctx.enter_context(tc.tile_pool(name="psum", bufs=2, space="PSUM"))
        psum2 = ctx.enter_context(tc.tile_pool(name="psum2", bufs=2, space="PSUM"))

        # input DMAs spread across queues for parallel issue
        xt_sb = singles.tile([128, T // 128 * BL], I32)
        nc.sync.dma_start(out=xt_sb[:], in_=x_t[:])
        tagsf_sb = singles.tile([BL, T], BF16)
        nc.scalar.dma_start(out=tagsf_sb[:], in_=tags_f[:])
        bcast8_sb = singles.tile([BL, 128], BF16)
        nc.scalar.dma_start(out=bcast8_sb[:], in_=bcast8[:])
        iotar_sb = singles.tile([128, CHW], BF16)
        nc.scalar.dma_start(out=iotar_sb[:], in_=iota_rep[:])
        blockP_sb = singles.tile([128, 128], BF16)
        blockPT_sb = singles.tile([128, 128], BF16)
        sadj_sb = singles.tile([128, 1], BF16)

        TM = big.tile([128, T], BF16)
        G = big.tile([128, T], BF16)
        EXPG = big.tile([128, T], BF16)
        W = big.tile([128, T], BF16)
        num_sb = singles.tile([128, 2 * NCHUNK], F32)

        nc.vector.memset(num_sb[:], 0.0)

        EXPGap = EXPG[:]
        Gap = G[:]

        # --- numerator W build first: only needs tags, runs during gather ---
        for c in range(NCHUNK):
            c0 = c * CHW
            psA = psum.tile([128, CHW], F32, tag="ps")
            nc.tensor.matmul(psA[:], lhsT=bcast8_sb[:],
                             rhs=tagsf_sb[:, c0:c0 + CHW], start=True, stop=True)
            nc.vector.tensor_tensor(out=W[:, c0:c0 + CHW], in0=psA[:],
                                    in1=iotar_sb[:], op=mybir.AluOpType.is_equal)

        # --- gather (token-major): one merged indirect DMA per chunk.
        # Offsets xt_sb[:, c*32:(c+1)*32] enumerate (partition, col) C-order;
        # each offset owns 16 contiguous bf16 of the dest view — identical
        # mapping to per-column calls but amortizes the ~1us SWDGE fixed cost.
        ncc = CHW // 16
        for c in range(NCHUNK):
            c0 = c * CHW
            nc.gpsimd.indirect_dma_start(
                out=TM[:, c0:c0 + CHW],
                out_offset=None,
                in_=t2[:],
                in_offset=bass.IndirectOffsetOnAxis(
                    ap=xt_sb[:, c * ncc:(c + 1) * ncc], axis=0),
            )
        # scan params: issued on the gpsimd queue AFTER the gather SWDGE so
        # they don't delay it; they complete long before the scan needs them
        nc.gpsimd.dma_start(out=blockP_sb[:], in_=blockP[:])
        nc.gpsimd.dma_start(out=blockPT_sb[:], in_=blockPT[:])
        nc.gpsimd.dma_start(out=sadj_sb[:], in_=sadj[:])

        # --- XBAR block-transpose TM -> G (2 chunks per call) + exp ---
        # out view (128, nb, 128): out[p, b, j] = TM[j, g0 + b*128 + p],
        # i.e. an independent transpose of each 128x128 block.
        GRPW = 2 * CHW
        nbl = GRPW // 128
        for g in range(NCHUNK // 2):
            g0 = g * GRPW
            eng = nc.sync if g % 2 == 0 else nc.scalar
            eng.dma_start_transpose(
                out=bass.AP(tensor=Gap.tensor, offset=g0,
                            ap=[[Gap.ap[0][0], 128], [128, nbl], [1, 128]]),
                in_=TM[:, g0:g0 + GRPW])
            nc.scalar.activation(EXPG[:, g0:g0 + GRPW], G[:, g0:g0 + GRPW],
                                 mybir.ActivationFunctionType.Exp)
        nc.vector.tensor_mul(EXPG[:, 0:1], EXPG[:, 0:1], sadj_sb[:])

        # --- numerator: em_tag only (transition terms done on host) ---
        for c in range(NCHUNK):
            c0 = c * CHW
            scr2 = scratch.tile([128, CHW], BF16, tag="scr2")
            nc.vector.tensor_mul(scr2[:], G[:, c0:c0 + CHW], W[:, c0:c0 + CHW])
            nc.vector.reduce_sum(out=num_sb[:, c:c + 1], in_=scr2[:],
                                 axis=mybir.AxisListType.X)

        # --- scans: forward and backward chains interleaved ---
        r_sb = big.tile([128, S], BF16)
        nc.vector.memset(r_sb[:], 1.0)
        d_sb = big.tile([128, S], BF16)
        nc.vector.tensor_copy(d_sb[:], _strided(EXPGap, L - 1, L, S))
        for i in range(L):
            kf = i               # forward step k = 0..15
            kb = L - 2 - i       # backward step k = 14..0
            psR = psum2.tile([128, S], F32, tag="psR")
            nc.tensor.matmul(psR[:], lhsT=blockP_sb[:], rhs=r_sb[:],
                             start=True, stop=True)
            if kb >= 0:
                psD = psum2.tile([128, S], F32, tag="psD")
                nc.tensor.matmul(psD[:], lhsT=blockPT_sb[:], rhs=d_sb[:],
                                 start=True, stop=True)
            nc.vector.tensor_mul(r_sb[:], psR[:], _strided(EXPGap, kf, L, S))
            if kb >= 0:
                nc.vector.tensor_mul(d_sb[:], psD[:], _strided(EXPGap, kb, L, S))

        nc.sync.dma_start(out=r_out[:], in_=r_sb[:])
        nc.sync.dma_start(out=d_out[:], in_=d_sb[:])
        nc.sync.dma_start(out=num_out[:], in_=num_sb[:])
    return nc


def _host_prep(embedding, fc_w, fc_b, trans, start):
    P_eff64 = np.exp(trans.astype(np.float64) + fc_b[None, :].astype(np.float64))
    colsum = P_eff64.sum(0)
    start_adj = (np.exp(start.astype(np.float64) + fc_b) / colsum).astype(np.float32)
    trans_n = (trans + fc_b[None, :]).astype(np.float32)
    P_eff32 = P_eff64.astype(np.float32)

    eye8 = np.eye(BL, dtype=np.float32)
    return dict(
        P_eff=P_eff64,
        trans_n=trans_n.astype(np.float64),
        blockP=np.ascontiguousarray(np.kron(eye8, P_eff32)).astype(NPBF),
        blockPT=np.ascontiguousarray(np.kron(eye8, P_eff32.T.copy())).astype(NPBF),
        bcast8=np.ascontiguousarray(np.kron(eye8, np.ones((1, C), np.float32))).astype(NPBF),
        iota_rep=np.ascontiguousarray(
            np.tile(np.tile(np.arange(C, dtype=np.float32), BL)[:, None],
                    (1, CHW))).astype(NPBF),
        sadj=np.ascontiguousarray(np.tile(start_adj, BL)[:, None]).astype(NPBF),
    )


LAST_RESULTS = {}


def _run(nc, in_maps, label):
    res = run_bass_kernel_spmd(nc, in_maps, core_ids=list(range(NCORES)),
                               trace=_TRACE)
    if res.exec_time_ns is not None:
        LAST_EXEC_NS[label] = res.exec_time_ns
    LAST_RESULTS[label] = res
    return res.results


def kernel(x, tags, embedding, fc_w, fc_b, start_transitions, end_transitions,
           transitions):
    x = np.asarray(x, np.int32)
    tags = np.asarray(tags, np.int32)
    embedding = np.asarray(embedding, np.float32)
    fc_w = np.asarray(fc_w, np.float32)
    fc_b = np.asarray(fc_b, np.float32)
    trans = np.asarray(transitions, np.float32)
    start = np.asarray(start_transitions, np.float32)
    end = np.asarray(end_transitions, np.float32)

    prep = _host_prep(embedding, fc_w, fc_b, trans, start)

    if "t2" not in _CACHE:
        nc1 = build_t2_kernel()
        nc1.finalize()
        _CACHE["t2"] = nc1
    if "main" not in _CACHE:
        nc2 = build_main_kernel()
        nc2.finalize()
        _CACHE["main"] = nc2

    # ---- launch 1: t2 = emb_pad @ fc_w (bf16 out), vocab-sharded ----
    emb_pad_T = np.zeros((E, VPAD), np.float32)
    emb_pad_T[:, :V] = embedding.T
    in1 = [{"embT_s": np.ascontiguousarray(emb_pad_T[:, k * VSH:(k + 1) * VSH]),
            "fc_w": fc_w} for k in range(NCORES)]
    res1 = _run(_CACHE["t2"], in1, "t2")
    t2_full = np.concatenate([np.asarray(res1[k]["t2_s"]) for k in range(NCORES)],
                             axis=0)
    t2_full = np.ascontiguousarray(t2_full)          # (VPAD, C) bf16

    # ---- launch 2: main kernel, batch-sharded ----
    tags_m = np.where(x != 0, tags, C).astype(NPBF)
    in2 = []
    for k in range(NCORES):
        sl = slice(k * BL, (k + 1) * BL)
        xt = x[sl].reshape(BL, T // 128, 128).transpose(2, 1, 0) \
                  .reshape(128, T // 128 * BL)
        in2.append({
            "x_t": np.ascontiguousarray(xt),
            "tags_f": np.ascontiguousarray(tags_m[sl]),
            "t2": t2_full,
            "blockP": prep["blockP"], "blockPT": prep["blockPT"],
            "bcast8": prep["bcast8"],
            "iota_rep": prep["iota_rep"], "sadj": prep["sadj"],
        })
    res2 = _run(_CACHE["main"], in2, "main")

    # ---- host combine (float64, vectorized) ----
    lengths = (x != 0).sum(1)                        # (B,)
    start64 = start.astype(np.float64)
    end64 = end.astype(np.float64)
    fcb64 = fc_b.astype(np.float64)
    Pe = prep["P_eff"]                               # (C, C) float64
    t264 = t2_full.astype(np.float64)                # (VPAD, C)
    exp_end = np.exp(end64)

    num_p = np.concatenate(
        [np.asarray(res2[k]["num_out"], np.float64) for k in range(NCORES)],
        axis=0).reshape(B, C, 2 * NCHUNK)            # (B, C, 16)
    r = np.concatenate(
        [np.asarray(res2[k]["r_out"]).astype(np.float64).reshape(BL, C, S)
         for k in range(NCORES)], axis=0)            # (B, C, S)
    d = np.concatenate(
        [np.asarray(res2[k]["d_out"]).astype(np.float64).reshape(BL, C, S)
         for k in range(NCORES)], axis=0)            # (B, C, S)

    num = num_p.sum(axis=(1, 2))
    num += start64[tags[:, 0]] + fcb64[tags[:, 0]]
    num += end64[tags[np.arange(B), lengths - 1]]
    # transition terms (pure tags/params, no device data)
    maskf = (x[:, 1:] != 0).astype(np.float64)
    num += (prep["trans_n"][tags[:, :-1], tags[:, 1:]] * maskf).sum(axis=1)

    # full-segment junction chain: for s in 1..sstar-1:
    #   logZ += log(r[:,:,s-1] @ (Pe @ d[:,:,s])) - log(r[:,:,s].sum())
    sstar = (lengths - 1) // L                       # (B,)
    cs = np.einsum('cd,bds->bcs', Pe, d)             # (B, C, S)
    t1 = np.einsum('bcs,bcs->bs', r[:, :, :-1], cs[:, :, 1:])   # junction at s=1..S-1
    rs = r.sum(axis=1)                               # (B, S)
    s_idx = np.arange(1, S)[None, :]                 # (1, S-1)
    jmask = s_idx <= (sstar[:, None] - 1)            # (B, S-1)
    terms = np.where(jmask, np.log(t1) - np.log(rs[:, 1:]), 0.0)
    logZ = terms.sum(axis=1)                         # (B,)

    # ragged tail: exact alpha recursion from segment sstar-1's r
    alpha = r[np.arange(B), :, sstar - 1].copy()     # (B, C)
    tail_len = lengths - sstar * L                   # in [1, L]
    for t_off in range(L):
        active = t_off < tail_len                    # (B,)
        t_idx = np.minimum(sstar * L + t_off, T - 1)
        w = np.exp(t264[x[np.arange(B), t_idx]] + fcb64[None, :])   # (B, C)
        alpha_new = (alpha @ Pe) * w
        alpha = np.where(active[:, None], alpha_new, alpha)
    logZ += np.log(alpha @ exp_end)

    total = -(num - logZ).sum()
    return np.array(total, dtype=np.float32)


# revision 17
# speedup vs baseline: 4.9770x; 1.0336x over previous
"""CRF negative-log-likelihood kernel for Trainium2 (8 NeuronCores, batch-sharded).

Algorithm:
  - Launch 1 (vocab-sharded): t2 = embedding @ fc_w in bf16. Host pre-transposes
    the embedding shard so the kernel is just convert-to-bf16 + 50 matmuls
    (lhsT = embT chunk, rhs = fc_w), no PE transposes. Output t2 is bf16
    (32B rows) to halve gather traffic.
  - Launch 2 (batch-sharded, 8 rows/core, bf16 compute): merged indirect-DMA
    gathers of t2 rows (8 calls, 4096 descriptors each), bf16 PE-block
    transposes into class-on-partition layout, numerator via one-hot matmul +
    multiply-reduce, and a segmented forward/backward scan (L=16 steps, S=256
    segments on the free dim) in linear space with the two scan chains
    interleaved so vector muls hide behind the other chain's matmuls.
  - Host (float64, vectorized): rank-1 junction chain across segments, exact
    partial segment for each row's ragged tail, final scalar assembly.
"""
import sys
sys.path.insert(0, "/opt/trn_rl_repo")
import numpy as np
import ml_dtypes
from contextlib import ExitStack

import concourse.bass as bass
import concourse.bacc as bacc_mod
import concourse.mybir as mybir
import concourse.tile as tile
from concourse.masks import make_identity
from concourse.bass_utils import run_bass_kernel_spmd

F32 = mybir.dt.float32
BF16 = mybir.dt.bfloat16
I32 = mybir.dt.int32
NPBF = ml_dtypes.bfloat16

V, E, C = 50257, 128, 16
B, T = 64, 4096
L, S = 8, 512
VPAD = 51200
VSH = VPAD // 8
BL = 8
NCHUNK = 8
CHW = T // NCHUNK
NCORES = 8

LAST_EXEC_NS = {}
_TRACE = False
_CACHE = {}


def build_t2_kernel():
    nc = bacc_mod.Bacc()
    # embT_s: host-pretransposed shard, (E, VSH) f32
    embT_s = nc.dram_tensor("embT_s", [E, VSH], F32, kind="ExternalInput")
    fc_w = nc.dram_tensor("fc_w", [E, C], F32, kind="ExternalInput")
    t2_s = nc.dram_tensor("t2_s", [VSH, C], BF16, kind="ExternalOutput")

    ntile = VSH // 128          # 50 chunks of 128 vocab rows
    NGRP = 10                   # DMA/convert granularity: 5 chunks per group
    GW = VSH // NGRP            # 640 columns per group
    with ExitStack() as ctx:
        tc = ctx.enter_context(tile.TileContext(nc))
        singles = ctx.enter_context(tc.tile_pool(name="singles", bufs=1))
        psum = ctx.enter_context(tc.tile_pool(name="psum", bufs=4, space="PSUM"))

        fcw_f32 = singles.tile([E, C], F32)
        nc.sync.dma_start(out=fcw_f32[:], in_=fc_w[:])
        fcw_bf = singles.tile([E, C], BF16)
        nc.vector.tensor_copy(fcw_bf[:], fcw_f32[:])

        EMBT = singles.tile([128, VSH], F32)
        EMBTb = singles.tile([128, VSH], BF16)
        T2 = singles.tile([128, ntile * C], BF16)
        for g in range(NGRP):
            nc.sync.dma_start(out=EMBT[:, g * GW:(g + 1) * GW],
                              in_=embT_s[:, g * GW:(g + 1) * GW])
            if g % 2 == 0:
                nc.vector.tensor_copy(EMBTb[:, g * GW:(g + 1) * GW],
                                      EMBT[:, g * GW:(g + 1) * GW])
            else:
                nc.scalar.copy(EMBTb[:, g * GW:(g + 1) * GW],
                               EMBT[:, g * GW:(g + 1) * GW])
        for i in range(ntile):
            ps2 = psum.tile([128, C], F32, tag="p2")
            nc.tensor.matmul(ps2[:], lhsT=EMBTb[:, i * 128:(i + 1) * 128],
                             rhs=fcw_bf[:], start=True, stop=True)
            if i % 2 == 0:
                nc.vector.tensor_copy(T2[:, i * C:(i + 1) * C], ps2[:])
            else:
                nc.scalar.copy(T2[:, i * C:(i + 1) * C], ps2[:])
        # one DMA out: (128, ntile*C) -> t2_s (VSH, C); dst dims (r, i, j)
        nc.sync.dma_start(
            out=bass.AP(tensor=t2_s[:].tensor, offset=0,
                        ap=[[C, 128], [128 * C, ntile], [1, C]]),
            in_=T2[:])
    return nc


def _strided(base_ap, k, step, count):
    return bass.AP(tensor=base_ap.tensor, offset=base_ap.offset + k,
                   ap=[base_ap.ap[0], [step, count]])


def build_main_kernel():
    nc = bacc_mod.Bacc()
    x_t = nc.dram_tensor("x_t", [128, T // 128 * BL], I32, kind="ExternalInput")
    tags_f = nc.dram_tensor("tags_f", [BL, T], BF16, kind="ExternalInput")
    t2 = nc.dram_tensor("t2", [VPAD, C], BF16, kind="ExternalInput")
    blockP = nc.dram_tensor("blockP", [128, 128], BF16, kind="ExternalInput")
    blockPT = nc.dram_tensor("blockPT", [128, 128], BF16, kind="ExternalInput")
    bcast8 = nc.dram_tensor("bcast8", [BL, 128], BF16, kind="ExternalInput")
    iota_rep = nc.dram_tensor("iota_rep", [128, CHW], BF16, kind="ExternalInput")
    sadj = nc.dram_tensor("sadj", [128, 1], BF16, kind="ExternalInput")

    r_out = nc.dram_tensor("r_out", [128, S], BF16, kind="ExternalOutput")
    d_out = nc.dram_tensor("d_out", [128, S], BF16, kind="ExternalOutput")
    num_out = nc.dram_tensor("num_out", [128, 2 * NCHUNK], F32, kind="ExternalOutput")

    with ExitStack() as ctx:
        tc = ctx.enter_context(tile.TileContext(nc))
        singles = ctx.enter_context(tc.tile_pool(name="singles", bufs=1))
        big = ctx.enter_context(tc.tile_pool(name="big", bufs=1))
        scratch = ctx.enter_context(tc.tile_pool(name="scratch", bufs=3))
        psum = ctx.enter_context(tc.tile_pool(name="psum", bufs=2, space="PSUM"))
        psum2 = ctx.enter_context(tc.tile_pool(name="psum2", bufs=2, space="PSUM"))

        # input DMAs spread across queues for parallel issue
        xt_sb = singles.tile([128, T // 128 * BL], I32)
        nc.sync.dma_start(out=xt_sb[:], in_=x_t[:])
        tagsf_sb = singles.tile([BL, T], BF16)
        nc.scalar.dma_start(out=tagsf_sb[:], in_=tags_f[:])
        bcast8_sb = singles.tile([BL, 128], BF16)
        nc.scalar.dma_start(out=bcast8_sb[:], in_=bcast8[:])
        iotar_sb = singles.tile([128, CHW], BF16)
        nc.scalar.dma_start(out=iotar_sb[:], in_=iota_rep[:])
        blockP_sb = singles.tile([128, 128], BF16)
        blockPT_sb = singles.tile([128, 128], BF16)
        sadj_sb = singles.tile([128, 1], BF16)

        TM = big.tile([128, T], BF16)
        G = big.tile([128, T], BF16)
        EXPG = big.tile([128, T], BF16)
        W = big.tile([128, T], BF16)
        num_sb = singles.tile([128, 2 * NCHUNK], F32)

        nc.vector.memset(num_sb[:], 0.0)

        EXPGap = EXPG[:]
        Gap = G[:]

        # --- numerator W build first: only needs tags, runs during gather ---
        for c in range(NCHUNK):
            c0 = c * CHW
            psA = psum.tile([128, CHW], F32, tag="ps")
            nc.tensor.matmul(psA[:], lhsT=bcast8_sb[:],
                             rhs=tagsf_sb[:, c0:c0 + CHW], start=True, stop=True)
            nc.vector.tensor_tensor(out=W[:, c0:c0 + CHW], in0=psA[:],
                                    in1=iotar_sb[:], op=mybir.AluOpType.is_equal)

        # --- gather (token-major): one merged indirect DMA per chunk.
        # Offsets xt_sb[:, c*32:(c+1)*32] enumerate (partition, col) C-order;
        # each offset owns 16 contiguous bf16 of the dest view — identical
        # mapping to per-column calls but amortizes the ~1us SWDGE fixed cost.
        ncc = CHW // 16
        for c in range(NCHUNK):
            c0 = c * CHW
            nc.gpsimd.indirect_dma_start(
                out=TM[:, c0:c0 + CHW],
                out_offset=None,
                in_=t2[:],
                in_offset=bass.IndirectOffsetOnAxis(
                    ap=xt_sb[:, c * ncc:(c + 1) * ncc], axis=0),
            )
        # scan params: issued on the gpsimd queue AFTER the gather SWDGE so
        # they don't delay it; they complete long before the scan needs them
        nc.gpsimd.dma_start(out=blockP_sb[:], in_=blockP[:])
        nc.gpsimd.dma_start(out=blockPT_sb[:], in_=blockPT[:])
        nc.gpsimd.dma_start(out=sadj_sb[:], in_=sadj[:])

        # --- XBAR block-transpose TM -> G (2 chunks per call) + exp ---
        # out view (128, nb, 128): out[p, b, j] = TM[j, g0 + b*128 + p],
        # i.e. an independent transpose of each 128x128 block.
        GRPW = 2 * CHW
        nbl = GRPW // 128
        for g in range(NCHUNK // 2):
            g0 = g * GRPW
            eng = nc.sync if g % 2 == 0 else nc.scalar
            eng.dma_start_transpose(
                out=bass.AP(tensor=Gap.tensor, offset=g0,
                            ap=[[Gap.ap[0][0], 128], [128, nbl], [1, 128]]),
                in_=TM[:, g0:g0 + GRPW])
            nc.scalar.activation(EXPG[:, g0:g0 + GRPW], G[:, g0:g0 + GRPW],
                                 mybir.ActivationFunctionType.Exp)
        nc.vector.tensor_mul(EXPG[:, 0:1], EXPG[:, 0:1], sadj_sb[:])

        # --- numerator: em_tag only (transition terms done on host) ---
        for c in range(NCHUNK):
            c0 = c * CHW
            scr2 = scratch.tile([128, CHW], BF16, tag="scr2")
            nc.vector.tensor_mul(scr2[:], G[:, c0:c0 + CHW], W[:, c0:c0 + CHW])
            nc.vector.reduce_sum(out=num_sb[:, c:c + 1], in_=scr2[:],
                                 axis=mybir.AxisListType.X)

        # --- scans: forward and backward chains interleaved ---
        r_sb = big.tile([128, S], BF16)
        nc.vector.memset(r_sb[:], 1.0)
        d_sb = big.tile([128, S], BF16)
        nc.vector.tensor_copy(d_sb[:], _strided(EXPGap, L - 1, L, S))
        for i in range(L):
            kf = i               # forward step k = 0..15
            kb = L - 2 - i       # backward step k = 14..0
            psR = psum2.tile([128, S], F32, tag="psR")
            nc.tensor.matmul(psR[:], lhsT=blockP_sb[:], rhs=r_sb[:],
                             start=True, stop=True)
            if kb >= 0:
                psD = psum2.tile([128, S], F32, tag="psD")
                nc.tensor.matmul(psD[:], lhsT=blockPT_sb[:], rhs=d_sb[:],
                                 start=True, stop=True)
            nc.vector.tensor_mul(r_sb[:], psR[:], _strided(EXPGap, kf, L, S))
            if kb >= 0:
                nc.vector.tensor_mul(d_sb[:], psD[:], _strided(EXPGap, kb, L, S))

        nc.sync.dma_start(out=r_out[:], in_=r_sb[:])
        nc.sync.dma_start(out=d_out[:], in_=d_sb[:])
        nc.sync.dma_start(out=num_out[:], in_=num_sb[:])
    return nc


def _host_prep(embedding, fc_w, fc_b, trans, start):
    P_eff64 = np.exp(trans.astype(np.float64) + fc_b[None, :].astype(np.float64))
    colsum = P_eff64.sum(0)
    start_adj = (np.exp(start.astype(np.float64) + fc_b) / colsum).astype(np.float32)
    trans_n = (trans + fc_b[None, :]).astype(np.float32)
    P_eff32 = P_eff64.astype(np.float32)

    eye8 = np.eye(BL, dtype=np.float32)
    return dict(
        P_eff=P_eff64,
        trans_n=trans_n.astype(np.float64),
        blockP=np.ascontiguousarray(np.kron(eye8, P_eff32)).astype(NPBF),
        blockPT=np.ascontiguousarray(np.kron(eye8, P_eff32.T.copy())).astype(NPBF),
        bcast8=np.ascontiguousarray(np.kron(eye8, np.ones((1, C), np.float32))).astype(NPBF),
        iota_rep=np.ascontiguousarray(
            np.tile(np.tile(np.arange(C, dtype=np.float32), BL)[:, None],
                    (1, CHW))).astype(NPBF),
        sadj=np.ascontiguousarray(np.tile(start_adj, BL)[:, None]).astype(NPBF),
    )


LAST_RESULTS = {}


def _run(nc, in_maps, label):
    res = run_bass_kernel_spmd(nc, in_maps, core_ids=list(range(NCORES)),
                               trace=_TRACE)
    if res.exec_time_ns is not None:
        LAST_EXEC_NS[label] = res.exec_time_ns
    LAST_RESULTS[label] = res
    return res.results


def kernel(x, tags, embedding, fc_w, fc_b, start_transitions, end_transitions,
           transitions):
    x = np.asarray(x, np.int32)
    tags = np.asarray(tags, np.int32)
    embedding = np.asarray(embedding, np.float32)
    fc_w = np.asarray(fc_w, np.float32)
    fc_b = np.asarray(fc_b, np.float32)
    trans = np.asarray(transitions, np.float32)
    start = np.asarray(start_transitions, np.float32)
    end = np.asarray(end_transitions, np.float32)

    prep = _host_prep(embedding, fc_w, fc_b, trans, start)

    if "t2" not in _CACHE:
        nc1 = build_t2_kernel()
        nc1.finalize()
        _CACHE["t2"] = nc1
    if "main" not in _CACHE:
        nc2 = build_main_kernel()
        nc2.finalize()
        _CACHE["main"] = nc2

    # ---- launch 1: t2 = emb_pad @ fc_w (bf16 out), vocab-sharded ----
    emb_pad_T = np.zeros((E, VPAD), np.float32)
    emb_pad_T[:, :V] = embedding.T
    in1 = [{"embT_s": np.ascontiguousarray(emb_pad_T[:, k * VSH:(k + 1) * VSH]),
            "fc_w": fc_w} for k in range(NCORES)]
    res1 = _run(_CACHE["t2"], in1, "t2")
    t2_full = np.concatenate([np.asarray(res1[k]["t2_s"]) for k in range(NCORES)],
                             axis=0)
    t2_full = np.ascontiguousarray(t2_full)          # (VPAD, C) bf16

    # ---- launch 2: main kernel, batch-sharded ----
    tags_m = np.where(x != 0, tags, C).astype(NPBF)
    in2 = []
    for k in range(NCORES):
        sl = slice(k * BL, (k + 1) * BL)
        xt = x[sl].reshape(BL, T // 128, 128).transpose(2, 1, 0) \
                  .reshape(128, T // 128 * BL)
        in2.append({
            "x_t": np.ascontiguousarray(xt),
            "tags_f": np.ascontiguousarray(tags_m[sl]),
            "t2": t2_full,
            "blockP": prep["blockP"], "blockPT": prep["blockPT"],
            "bcast8": prep["bcast8"],
            "iota_rep": prep["iota_rep"], "sadj": prep["sadj"],
        })
    res2 = _run(_CACHE["main"], in2, "main")

    # ---- host combine (float64, vectorized) ----
    lengths = (x != 0).sum(1)                        # (B,)
    start64 = start.astype(np.float64)
    end64 = end.astype(np.float64)
    fcb64 = fc_b.astype(np.float64)
    Pe = prep["P_eff"]                               # (C, C) float64
    t264 = t2_full.astype(np.float64)                # (VPAD, C)
    exp_end = np.exp(end64)

    num_p = np.concatenate(
        [np.asarray(res2[k]["num_out"], np.float64) for k in range(NCORES)],
        axis=0).reshape(B, C, 2 * NCHUNK)            # (B, C, 16)
    r = np.concatenate(
        [np.asarray(res2[k]["r_out"]).astype(np.float64).reshape(BL, C, S)
         for k in range(NCORES)], axis=0)            # (B, C, S)
    d = np.concatenate(
        [np.asarray(res2[k]["d_out"]).astype(np.float64).reshape(BL, C, S)
         for k in range(NCORES)], axis=0)            # (B, C, S)

    num = num_p.sum(axis=(1, 2))
    num += start64[tags[:, 0]] + fcb64[tags[:, 0]]
    num += end64[tags[np.arange(B), lengths - 1]]
    # transition terms (pure tags/params, no device data)
    maskf = (x[:, 1:] != 0).astype(np.float64)
    num += (prep["trans_n"][tags[:, :-1], tags[:, 1:]] * maskf).sum(axis=1)

    # full-segment junction chain: for s in 1..sstar-1:
    #   logZ += log(r[:,:,s-1] @ (Pe @ d[:,:,s])) - log(r[:,:,s].sum())
    sstar = (lengths - 1) // L                       # (B,)
    cs = np.einsum('cd,bds->bcs', Pe, d)             # (B, C, S)
    t1 = np.einsum('bcs,bcs->bs', r[:, :, :-1], cs[:, :, 1:])   # junction at s=1..S-1
    rs = r.sum(axis=1)                               # (B, S)
    s_idx = np.arange(1, S)[None, :]                 # (1, S-1)
    jmask = s_idx <= (sstar[:, None] - 1)            # (B, S-1)
    terms = np.where(jmask, np.log(t1) - np.log(rs[:, 1:]), 0.0)
    logZ = terms.sum(axis=1)                         # (B,)

    # ragged tail: exact alpha recursion from segment sstar-1's r
    alpha = r[np.arange(B), :, sstar - 1].copy()     # (B, C)
    tail_len = lengths - sstar * L                   # in [1, L]
    for t_off in range(L):
        active = t_off < tail_len                    # (B,)
        t_idx = np.minimum(sstar * L + t_off, T - 1)
        w = np.exp(t264[x[np.arange(B), t_idx]] + fcb64[None, :])   # (B, C)
        alpha_new = (alpha @ Pe) * w
        alpha = np.where(active[:, None], alpha_new, alpha)
    logZ += np.log(alpha @ exp_end)

    total = -(num - logZ).sum()
    return np.array(total, dtype=np.float32)
